# revision 4
# baseline (speedup 1.0000x reference)
# LoRA-MoE QK kernel for 8x Trainium2 NeuronCores (Bass/Tile).
#
# Reference computation:
#   routing = softmax(mean(x[:, 611:-1, :]) @ router_W.T + router_b)   [B, E]
#   base    = x @ W.T + b
#   lora    = einsum('bsd,erd->bser', x, A) -> *B,routing -> [B,S,O] * 2.0
#   out     = base + lora
#
# Sharding: data-parallel over the 8192 tokens (1024/core; each core's tokens
# belong to exactly one batch; a batch spans cores {2b, 2b+1}).  Weights
# replicated, host-prepped (bf16 cast + transpose); router computed on host.
#
# v2 changes vs baseline (530us):
#  - DMA issue order == consumption order: afT first, then per-k interleaved
#    {x k-slice, w-panel-0 k-slice}, then bfT/bias, then later panels.  The
#    baseline issued all of x before afT, so the first (LoRA-t) matmul waited
#    for the full 8MB x load (first MM at t=38us).
#  - Panel 0 runs k-outer with 6 concurrent token PSUM groups + the 2 LoRA-t
#    groups (all 8 PSUM banks), giving the PE ~1.7us of work per arriving
#    k-slice so it stays busy through the DMA-fed startup window.
#  - fp16 output (halves output traffic, trims the post-compute tail;
#    values are O(10) so fp16 rounding is ~1e-4 relative).

import numpy as np
import ml_dtypes

BF16 = ml_dtypes.bfloat16

B_, S, D, O, E, R = 4, 2048, 4096, 4096, 8, 16
ER = E * R              # 128
TOK = B_ * S            # 8192
NCORES = 8
TPC = TOK // NCORES     # 1024 tokens per core
KT = D // 128           # 32 contraction tiles
NOB = O // 512          # 8 output-column panels
NTT = TPC // 128        # 8 token tiles per core
Q_LO, Q_HI = 611, 2047  # question tokens [611, 2047) within each batch

_CACHE: dict = {}
LAST_RESULTS = None
TRACE = False


def _build_nc():
    import concourse.bacc as bacc
    import concourse.mybir as mybir
    from concourse import tile

    fp32 = mybir.dt.float32
    fp16 = mybir.dt.float16
    bf16 = mybir.dt.bfloat16

    nc = bacc.Bacc(
        "TRN2",
        target_bir_lowering=False,
        debug=False,
        num_devices=NCORES,
    )

    xT = nc.dram_tensor("xT", [D, TPC], bf16, kind="ExternalInput")
    wT = nc.dram_tensor("wT", [D, O], bf16, kind="ExternalInput")
    afT = nc.dram_tensor("afT", [D, ER], bf16, kind="ExternalInput")
    bfT = nc.dram_tensor("bfT", [ER, O], bf16, kind="ExternalInput")
    biasrep = nc.dram_tensor("biasrep", [128, O], bf16, kind="ExternalInput")
    svec = nc.dram_tensor("svec", [128, 1], fp32, kind="ExternalInput")
    out = nc.dram_tensor("out", [TPC, O], fp16, kind="ExternalOutput")

    NG0 = 6  # token groups held open during the k-outer panel-0 pass

    with tile.TileContext(nc) as tc:
        with (
            tc.tile_pool(name="const", bufs=1) as const,
            tc.tile_pool(name="w", bufs=2 * KT) as wpool,
            tc.tile_pool(name="ot", bufs=4) as otpool,
            tc.tile_pool(name="po", bufs=NG0, space="PSUM") as po_pool,
            tc.tile_pool(name="pt", bufs=2, space="PSUM") as pt_pool,
        ):
            # ---- resident SBUF tensors ----
            xt_sb = const.tile([128, KT * TPC], bf16)      # [p, (k t)]
            afT_sb = const.tile([128, KT * ER], bf16)      # [p, (k er)]
            bfT_sb = const.tile([128, O], bf16)            # [er, o]
            biasrep_sb = const.tile([128, O], bf16)
            svec_sb = const.tile([128, 1], fp32)
            u_sb = const.tile([128, TPC], bf16)            # [er, t]

            # ---- priority DMAs: what the PE needs first ----
            nc.sync.dma_start(svec_sb[:], svec[:])
            for k in range(KT):
                nc.sync.dma_start(
                    afT_sb[:, k * ER:(k + 1) * ER], afT[k * 128:(k + 1) * 128, :]
                )
            # per-k interleave of x slice + panel-0 w slice so the k-outer
            # compute below can consume them in arrival order
            w0 = [wpool.tile([128, 512], bf16, tag="w", name=f"w0_{k}") for k in range(KT)]
            for k in range(KT):
                nc.sync.dma_start(
                    xt_sb[:, k * TPC:(k + 1) * TPC], xT[k * 128:(k + 1) * 128, :]
                )
                nc.sync.dma_start(w0[k][:], wT[k * 128:(k + 1) * 128, 0:512])
            # needed at first group-close (~60us in)
            nc.sync.dma_start(bfT_sb[:], bfT[:])
            for kk in range(4):
                nc.sync.dma_start(
                    biasrep_sb[:, kk * 1024:(kk + 1) * 1024],
                    biasrep[:, kk * 1024:(kk + 1) * 1024],
                )
            # prefetch panel 1 (second half of the double buffer is free)
            w1 = [wpool.tile([128, 512], bf16, tag="w", name=f"w1_{k}") for k in range(KT)]
            for k in range(KT):
                nc.sync.dma_start(w1[k][:], wT[k * 128:(k + 1) * 128, 512:1024])

            # ---- panel 0, k-outer: LoRA-t groups + NG0 token groups ----
            pt_tiles = [pt_pool.tile([128, 512], fp32, tag="pt", name=f"pt_{i}") for i in range(2)]
            po0 = [po_pool.tile([128, 512], fp32, tag="po", name=f"po0_{i}") for i in range(NG0)]
            for k in range(KT):
                for tb in range(2):
                    nc.tensor.matmul(
                        pt_tiles[tb][:],
                        afT_sb[:, k * ER:(k + 1) * ER],
                        xt_sb[:, k * TPC + tb * 512: k * TPC + tb * 512 + 512],
                        start=(k == 0),
                        stop=(k == KT - 1),
                    )
                for tt in range(NG0):
                    nc.tensor.matmul(
                        po0[tt][:],
                        xt_sb[:, k * TPC + tt * 128: k * TPC + tt * 128 + 128],
                        w0[k][:],
                        start=(k == 0),
                        stop=False,
                    )

            # u = t * routing (per-partition scalar), bf16
            for tb in range(2):
                nc.vector.tensor_scalar_mul(
                    u_sb[:, tb * 512:(tb + 1) * 512],
                    pt_tiles[tb][:],
                    svec_sb[:, 0:1],
                )

            def close_and_drain(po, tt, ob):
                nc.tensor.matmul(
                    po[:],
                    u_sb[:, tt * 128:(tt + 1) * 128],
                    bfT_sb[:, ob * 512:(ob + 1) * 512],
                    start=False,
                    stop=True,
                )
                ot = otpool.tile([128, 512], fp16)
                nc.vector.tensor_add(
                    ot[:], po[:], biasrep_sb[:, ob * 512:(ob + 1) * 512]
                )
                nc.sync.dma_start(
                    out[tt * 128:(tt + 1) * 128, ob * 512:(ob + 1) * 512],
                    ot[:],
                )

            for tt in range(NG0):
                close_and_drain(po0[tt], tt, 0)
            # remaining token tiles of panel 0, tt-outer (x is resident now)
            for tt in range(NG0, NTT):
                po = po_pool.tile([128, 512], fp32, tag="po", name="po")
                for k in range(KT):
                    nc.tensor.matmul(
                        po[:],
                        xt_sb[:, k * TPC + tt * 128: k * TPC + tt * 128 + 128],
                        w0[k][:],
                        start=(k == 0),
                        stop=False,
                    )
                close_and_drain(po, tt, 0)

            # ---- panels 1..7: as baseline (double-buffered w) ----
            wt_cur = w1
            for ob in range(1, NOB):
                if ob + 1 < NOB:
                    wt_next = [
                        wpool.tile([128, 512], bf16, tag="w", name=f"w{ob+1}_{k}")
                        for k in range(KT)
                    ]
                    for k in range(KT):
                        nc.sync.dma_start(
                            wt_next[k][:],
                            wT[k * 128:(k + 1) * 128,
                               (ob + 1) * 512:(ob + 2) * 512],
                        )
                else:
                    wt_next = None
                for tt in range(NTT):
                    po = po_pool.tile([128, 512], fp32, tag="po", name="po")
                    for k in range(KT):
                        nc.tensor.matmul(
                            po[:],
                            xt_sb[:, k * TPC + tt * 128: k * TPC + tt * 128 + 128],
                            wt_cur[k][:],
                            start=(k == 0),
                            stop=False,
                        )
                    close_and_drain(po, tt, ob)
                wt_cur = wt_next

    nc.compile()
    return nc


def _host_prep(x, W, b, A, B, router_W, router_b):
    xf = np.ascontiguousarray(x, dtype=np.float32).reshape(TOK, D)
    xT_bf = xf.T.astype(BF16)                       # [D, TOK]
    wT_bf = W.T.astype(BF16)                        # [D, O]
    afT_bf = A.reshape(ER, D).T.astype(BF16)        # [D, ER]
    bfT_bf = (2.0 * np.transpose(B, (0, 2, 1)).reshape(ER, O)).astype(BF16)
    bias_bf = np.ascontiguousarray(
        np.broadcast_to(b.astype(BF16)[None, :], (128, O))
    )
    # router on host (numpy, float64 — exact vs bf16 device noise)
    xq = np.asarray(x, np.float64)[:, Q_LO:Q_HI, :]
    q = xq.mean(axis=1)
    logits = q @ np.asarray(router_W, np.float64).T + np.asarray(router_b, np.float64)
    ex = np.exp(logits - logits.max(-1, keepdims=True))
    routing = ex / ex.sum(-1, keepdims=True)          # [B, E]

    shards = [
        np.ascontiguousarray(xT_bf[:, c * TPC:(c + 1) * TPC]) for c in range(NCORES)
    ]
    in_maps = []
    for c in range(NCORES):
        sv = np.repeat(routing[c // 2].astype(np.float32), R).reshape(128, 1)
        in_maps.append({
            "xT": shards[c],
            "wT": wT_bf,
            "afT": afT_bf,
            "bfT": bfT_bf,
            "biasrep": bias_bf,
            "svec": np.ascontiguousarray(sv),
        })
    return in_maps


def kernel(x, W, b, A, B, router_W, router_b):
    global LAST_RESULTS
    from concourse.bass_utils import run_bass_kernel_spmd

    if "nc" not in _CACHE:
        _CACHE["nc"] = _build_nc()
    nc = _CACHE["nc"]

    in_maps = _host_prep(x, W, b, A, B, router_W, router_b)

    kwargs = {}
    if TRACE:
        kwargs.update(trace=True, trace_cores=list(range(NCORES)))
    res = run_bass_kernel_spmd(nc, in_maps, core_ids=list(range(NCORES)), **kwargs)
    LAST_RESULTS = res

    shards = [res.results[c]["out"] for c in range(NCORES)]
    return np.concatenate(shards, axis=0).reshape(B_, S, O).astype(np.float32)


# revision 5
# speedup vs baseline: 1.2339x; 1.2339x over previous
# LoRA-MoE QK kernel for 8x Trainium2 NeuronCores (Bass/Tile).
#
# Reference computation:
#   routing = softmax(mean(x[:, 611:-1, :]) @ router_W.T + router_b)   [B, E]
#   base    = x @ W.T + b
#   lora    = einsum('bsd,erd->bser', x, A) -> *B,routing -> [B,S,O] * 2.0
#   out     = base + lora
#
# Sharding: data-parallel over the 8192 tokens (1024/core; each core's tokens
# belong to exactly one batch; a batch spans cores {2b, 2b+1}).  Weights
# replicated, host-prepped; router computed on host.
#
# v3 changes vs the 530us baseline:
#  - All inputs are host-swizzled into their exact SBUF images so every DMA
#    copy moves large contiguous per-partition rows.  The DMA engines process
#    one descriptor per partition row (~85ns each, ~190/us aggregate), so the
#    baseline's [128-row x 256B..2KB] strided loads were descriptor-rate
#    bound: the first matmul could not start until t=38us.  Swizzled images
#    cut the kernel's descriptor count from ~56k to ~12k and make startup
#    byte-bound instead (first MM ~11us).
#  - DMA issue order == consumption order (x/w panel-0 k-group 0 first, then
#    afT, remaining x/w0 groups, then bfT/bias and later panels).
#  - Panel 0 runs k-outer with 6 concurrent token PSUM groups + the 2 LoRA-t
#    groups (all 8 PSUM banks) so the PE has work as each k-group arrives.
#  - W panels 1..7 are single whole-panel DMAs (128 descriptors of 32KB).
#  - fp16 output (halves output traffic; values are O(10), fp16 rounding
#    ~1e-4 relative).

import numpy as np
import ml_dtypes

BF16 = ml_dtypes.bfloat16

B_, S, D, O, E, R = 4, 2048, 4096, 4096, 8, 16
ER = E * R              # 128
TOK = B_ * S            # 8192
NCORES = 8
TPC = TOK // NCORES     # 1024 tokens per core
KT = D // 128           # 32 contraction tiles
NOB = O // 512          # 8 output-column panels
NTT = TPC // 128        # 8 token tiles per core
Q_LO, Q_HI = 611, 2047  # question tokens [611, 2047) within each batch

XG = 4                  # x k-tiles per DMA group (8 groups of 1MB)
NG0 = 6                 # token groups held open during the k-outer panel-0 pass

_CACHE: dict = {}
LAST_RESULTS = None
TRACE = False


def _build_nc():
    import concourse.bacc as bacc
    import concourse.mybir as mybir
    from concourse import tile

    fp32 = mybir.dt.float32
    fp16 = mybir.dt.float16
    bf16 = mybir.dt.bfloat16

    nc = bacc.Bacc(
        "TRN2",
        target_bir_lowering=False,
        debug=False,
        num_devices=NCORES,
    )

    # all device inputs are pre-swizzled SBUF images (partition-major)
    xswz = nc.dram_tensor("xswz", [128, KT * TPC], bf16, kind="ExternalInput")
    wswz = nc.dram_tensor("wswz", [128, NOB * KT * 512], bf16, kind="ExternalInput")
    afswz = nc.dram_tensor("afswz", [128, KT * ER], bf16, kind="ExternalInput")
    bfT = nc.dram_tensor("bfT", [ER, O], bf16, kind="ExternalInput")
    biasrep = nc.dram_tensor("biasrep", [128, O], bf16, kind="ExternalInput")
    svec = nc.dram_tensor("svec", [128, 1], fp32, kind="ExternalInput")
    out = nc.dram_tensor("out", [TPC, O], fp16, kind="ExternalOutput")

    PW = KT * 512           # panel width in SBUF columns (16384)

    with tile.TileContext(nc) as tc:
        with (
            tc.tile_pool(name="const", bufs=1) as const,
            tc.tile_pool(name="w", bufs=2) as wpool,
            tc.tile_pool(name="ot", bufs=6) as otpool,
            tc.tile_pool(name="po", bufs=NG0, space="PSUM") as po_pool,
            tc.tile_pool(name="pt", bufs=2, space="PSUM") as pt_pool,
        ):
            # ---- resident SBUF tensors ----
            xt_sb = const.tile([128, KT * TPC], bf16)      # [p, (k t)]
            afT_sb = const.tile([128, KT * ER], bf16)      # [p, (k er)]
            bfT_sb = const.tile([128, O], bf16)            # [er, o]
            biasrep_sb = const.tile([128, O], bf16)
            svec_sb = const.tile([128, 1], fp32)
            u_sb = const.tile([128, TPC], bf16)            # [er, t]

            w0t = wpool.tile([128, PW], bf16, tag="w", name="w0")

            # ---- priority DMAs, in consumption order ----
            nc.sync.dma_start(svec_sb[:], svec[:])
            # x group 0 + w0 group 0 gate the first matmuls
            nc.sync.dma_start(xt_sb[:, 0:XG * TPC], xswz[:, 0:XG * TPC])
            nc.sync.dma_start(w0t[:, 0:XG * 512], wswz[:, 0:XG * 512])
            nc.sync.dma_start(afT_sb[:], afswz[:])
            for g in range(1, KT // XG):
                nc.sync.dma_start(
                    xt_sb[:, g * XG * TPC:(g + 1) * XG * TPC],
                    xswz[:, g * XG * TPC:(g + 1) * XG * TPC],
                )
                nc.sync.dma_start(
                    w0t[:, g * XG * 512:(g + 1) * XG * 512],
                    wswz[:, g * XG * 512:(g + 1) * XG * 512],
                )
            # needed at first group-close (~60us in)
            nc.sync.dma_start(bfT_sb[:], bfT[:])
            nc.sync.dma_start(biasrep_sb[:], biasrep[:])
            # prefetch panel 1
            w1t = wpool.tile([128, PW], bf16, tag="w", name="w1")
            nc.sync.dma_start(w1t[:], wswz[:, PW:2 * PW])

            # ---- panel 0, k-outer: NG0 token groups + LoRA-t groups ----
            pt_tiles = [
                pt_pool.tile([128, 512], fp32, tag="pt", name=f"pt_{i}")
                for i in range(2)
            ]
            po0 = [
                po_pool.tile([128, 512], fp32, tag="po", name=f"po0_{i}")
                for i in range(NG0)
            ]
            for k in range(KT):
                # po MMs first: they only need x+w (afT lands a bit later)
                for tt in range(NG0):
                    nc.tensor.matmul(
                        po0[tt][:],
                        xt_sb[:, k * TPC + tt * 128: k * TPC + tt * 128 + 128],
                        w0t[:, k * 512:(k + 1) * 512],
                        start=(k == 0),
                        stop=False,
                    )
                for tb in range(2):
                    nc.tensor.matmul(
                        pt_tiles[tb][:],
                        afT_sb[:, k * ER:(k + 1) * ER],
                        xt_sb[:, k * TPC + tb * 512: k * TPC + tb * 512 + 512],
                        start=(k == 0),
                        stop=(k == KT - 1),
                    )

            # u = t * routing (per-partition scalar), bf16
            for tb in range(2):
                nc.vector.tensor_scalar_mul(
                    u_sb[:, tb * 512:(tb + 1) * 512],
                    pt_tiles[tb][:],
                    svec_sb[:, 0:1],
                )

            def close_and_drain(po, tt, ob):
                nc.tensor.matmul(
                    po[:],
                    u_sb[:, tt * 128:(tt + 1) * 128],
                    bfT_sb[:, ob * 512:(ob + 1) * 512],
                    start=False,
                    stop=True,
                )
                ot = otpool.tile([128, 512], fp16)
                nc.vector.tensor_add(
                    ot[:], po[:], biasrep_sb[:, ob * 512:(ob + 1) * 512]
                )
                nc.sync.dma_start(
                    out[tt * 128:(tt + 1) * 128, ob * 512:(ob + 1) * 512],
                    ot[:],
                )

            for tt in range(NG0):
                close_and_drain(po0[tt], tt, 0)
            # remaining token tiles of panel 0, tt-outer (x is resident now)
            for tt in range(NG0, NTT):
                po = po_pool.tile([128, 512], fp32, tag="po", name="po")
                for k in range(KT):
                    nc.tensor.matmul(
                        po[:],
                        xt_sb[:, k * TPC + tt * 128: k * TPC + tt * 128 + 128],
                        w0t[:, k * 512:(k + 1) * 512],
                        start=(k == 0),
                        stop=False,
                    )
                close_and_drain(po, tt, 0)

            # ---- panels 1..7 (double-buffered whole-panel w DMAs) ----
            wt_cur = w1t
            for ob in range(1, NOB):
                if ob + 1 < NOB:
                    wt_next = wpool.tile(
                        [128, PW], bf16, tag="w", name=f"w{ob + 1}"
                    )
                    nc.sync.dma_start(
                        wt_next[:], wswz[:, (ob + 1) * PW:(ob + 2) * PW]
                    )
                else:
                    wt_next = None
                for tt in range(NTT):
                    po = po_pool.tile([128, 512], fp32, tag="po", name="po")
                    for k in range(KT):
                        nc.tensor.matmul(
                            po[:],
                            xt_sb[:, k * TPC + tt * 128: k * TPC + tt * 128 + 128],
                            wt_cur[:, k * 512:(k + 1) * 512],
                            start=(k == 0),
                            stop=False,
                        )
                    close_and_drain(po, tt, ob)
                wt_cur = wt_next

    nc.compile()
    return nc


def _host_prep(x, W, b, A, B, router_W, router_b):
    xf = np.ascontiguousarray(x, dtype=np.float32).reshape(TOK, D)
    # per-core SBUF image of x: xswz[p, k*TPC + t] = x[c*TPC + t, k*128 + p]
    xswz_cores = []
    for c in range(NCORES):
        xc = xf[c * TPC:(c + 1) * TPC].astype(BF16)          # [TPC, D]
        img = np.ascontiguousarray(
            xc.reshape(TPC, KT, 128).transpose(2, 1, 0)
        ).reshape(128, KT * TPC)
        xswz_cores.append(img)
    # W image: wswz[p, (ob*KT + k)*512 + j] = W[ob*512 + j, k*128 + p]
    wswz = np.ascontiguousarray(
        W.astype(BF16).reshape(NOB, 512, KT, 128).transpose(3, 0, 2, 1)
    ).reshape(128, NOB * KT * 512)
    # A image: afswz[p, k*ER + e] = A_flat[e, k*128 + p]
    afswz = np.ascontiguousarray(
        A.reshape(ER, D).astype(BF16).reshape(ER, KT, 128).transpose(2, 1, 0)
    ).reshape(128, KT * ER)
    bfT_bf = (2.0 * np.transpose(B, (0, 2, 1)).reshape(ER, O)).astype(BF16)
    bias_bf = np.ascontiguousarray(
        np.broadcast_to(b.astype(BF16)[None, :], (128, O))
    )
    # router on host (numpy, float64 — exact vs bf16 device noise)
    xq = np.asarray(x, np.float64)[:, Q_LO:Q_HI, :]
    q = xq.mean(axis=1)
    logits = q @ np.asarray(router_W, np.float64).T + np.asarray(router_b, np.float64)
    ex = np.exp(logits - logits.max(-1, keepdims=True))
    routing = ex / ex.sum(-1, keepdims=True)          # [B, E]

    in_maps = []
    for c in range(NCORES):
        sv = np.repeat(routing[c // 2].astype(np.float32), R).reshape(128, 1)
        in_maps.append({
            "xswz": xswz_cores[c],
            "wswz": wswz,
            "afswz": afswz,
            "bfT": bfT_bf,
            "biasrep": bias_bf,
            "svec": np.ascontiguousarray(sv),
        })
    return in_maps


def kernel(x, W, b, A, B, router_W, router_b):
    global LAST_RESULTS
    from concourse.bass_utils import run_bass_kernel_spmd

    if "nc" not in _CACHE:
        _CACHE["nc"] = _build_nc()
    nc = _CACHE["nc"]

    in_maps = _host_prep(x, W, b, A, B, router_W, router_b)

    kwargs = {}
    if TRACE:
        kwargs.update(trace=True, trace_cores=list(range(NCORES)))
    res = run_bass_kernel_spmd(nc, in_maps, core_ids=list(range(NCORES)), **kwargs)
    LAST_RESULTS = res

    shards = [res.results[c]["out"] for c in range(NCORES)]
    return np.concatenate(shards, axis=0).reshape(B_, S, O).astype(np.float32)
